# revision 25
# baseline (speedup 1.0000x reference)
"""CPAMDec attention-decoder kernel for 8 Trainium2 NeuronCores.

Reference computation (per batch n of N=8):
    q  = x_n^T @ wq^T + bq          (HW=4096, C4=128)
    k  = y_n @ wk^T + bk            (K=32, C4=128)
    v  = y_n @ wv^T + bv            (K=32, C=512)
    attn = softmax(q @ k^T, axis=-1)        (HW, K)
    out = scale * (v^T @ attn^T) + x_n      (C, HW)

Sharding: pure data parallel - core i computes batch i.

Everything that does not involve x (8MB/core) is precomputed on the host
in fp32 and shipped as two small consts (~170KB/core):
  - EM[c,j] = sum_o wq[o,c]*(k^T[o,j]+bk-bias'd)  so that per chunk
    e[j,p] = sum_c EM[c,j] x[c,p] + e_b[j]; e_b = bq @ ktb (exp bias).
  - vstack: scale*v (+ scale*bv baked into every row, valid since
    sum_j attn = 1) in the partition-stacked layout the 32-row
    tile_position matmuls want.

Device loop per 512-pixel chunk (bf16 end to end, rel-err ~2.7e-3 vs
the 2e-2 gate; HBM-per-NC ~358 GB/s is the roofline):
    e = EM^T x (4 mm) -> exp (ACT, bias e_b) -> sum via ones-matmul
    -> reciprocal (DVE) -> attn = expt*rec (Pool) -> 4x 32-row matmuls
    (PSUM, double-bank pairs) -> o_ps + x adds (DVE) -> store.
PE warm-up dummies ramp the HAM clock gate (1.2 -> 2.4 GHz after ~3.4us
of sustained execution) while the first DMAs land.
"""

import sys

sys.path.insert(0, "/opt/trn_rl_repo")

import numpy as np
import ml_dtypes

import concourse.bacc as bacc
import concourse.mybir as mybir
import concourse.tile as tile
from concourse.alu_op_type import AluOpType
from concourse.bass_utils import run_bass_kernel_spmd

F32 = mybir.dt.float32
BF16 = mybir.dt.bfloat16
AF = mybir.ActivationFunctionType
BF = ml_dtypes.bfloat16

N, C, H, W, K = 8, 512, 64, 64, 32
HW = H * W            # 4096
C4 = C // 4           # 128
PC = 512              # free-dim chunk (1 PSUM bank of fp32)
NPC = HW // PC        # 8 chunks
KC = C // 128         # 4 contraction chunks
CT = C // 128         # 4 output row-tiles


def _emit(nc, tc):
    sync = nc.sync

    with (
        tc.tile_pool(name="const", bufs=1) as cst,
        tc.tile_pool(name="xbuf", bufs=1) as xp,
        tc.tile_pool(name="work", bufs=3) as wk_pool,
        tc.tile_pool(name="ps", bufs=2, space="PSUM") as ps,
    ):
        # ---- consts first on the SP ring: FIFO beats the x flood ----
        # pem = EM as [512,128] -> [128, KC, 128]      bf16
        # pvs = vstack[128,128] | e_b (f32 as 2 bf16)  bf16
        pem = cst.tile([128, KC, 128], BF16, name="pem", tag="pem")
        sync.dma_start(pem[:],
                       nc.t.pem[:].rearrange("(k p) f -> p k f", p=128))
        pvs = cst.tile([128, 130], BF16, name="pvs", tag="pvs")
        sync.dma_start(pvs[:], nc.t.pvs[:])
        vstack = pvs[:, 0:128]
        e_b4 = pvs[:, 128:130].bitcast(F32)

        ones32 = cst.tile([K, 128], BF16, name="ones32", tag="ones32")
        nc.gpsimd.memset(ones32[:], 1.0)

        # x column chunks: (128 part, 4 c-tiles, PC cols) strided loads on
        # the SP ring, behind the consts. SBUF keeps all 8 resident.
        xs = [None] * NPC

        def load_chunk(pc):
            t = xp.tile([128, KC, PC], BF16, name=f"xs{pc}", tag=f"xs{pc}")
            src = nc.t.x[:, pc * PC:(pc + 1) * PC].rearrange(
                "(k p) f -> p k f", p=128)
            sync.dma_start(t[:], src)
            xs[pc] = t

        for pc in range(4):
            load_chunk(pc)

        # ---- PE warm-up: ramp the HAM clock gate while DMAs land ----
        dmy_ps = ps.tile([128, PC], F32, name="dmy_ps", tag="s", bufs=2)
        for _ in range(1):
            nc.tensor.matmul(dmy_ps[:], pem[:, 0, :], pem[:],
                             start=True, stop=True)

        # Load the exp ACT table before steady state (Copy/Identity live in
        # every table, so this is the only table load).
        acttbl = cst.tile([128, 8], BF16, name="acttbl", tag="acttbl")
        nc.scalar.activation(out=acttbl[:], in_=pem[:, 0, 0:8], func=AF.Exp,
                             bias=0.0, scale=1.0)

        # ------------- software-pipelined loop over column chunks ----
        #   step i:  e/exp(i)   sum/rec/mul(i-1)   out-mm/add/store(i-2)
        expts = [None] * NPC
        attns = [None] * NPC

        def stage_e(pc):
            e_ps = ps.tile([128, PC], F32, name=f"e_ps{pc}", tag="e", bufs=2)
            for k in range(KC):
                nc.tensor.matmul(e_ps[:], pem[:, k, :], xs[pc][:, k, :],
                                 start=(k == 0), stop=(k == KC - 1))
            expt = wk_pool.tile([128, PC], BF16, name="expt", tag="expt",
                                bufs=3)
            nc.scalar.activation(out=expt[:], in_=e_ps[:], func=AF.Exp,
                                 bias=e_b4[:], scale=1.0)
            expts[pc] = expt

        def stage_s(pc):
            s_ps = ps.tile([128, PC], F32, name=f"s_ps{pc}", tag="s", bufs=2)
            nc.tensor.matmul(s_ps[:], ones32[:], expts[pc][0:K, :],
                             start=True, stop=True)
            rec = wk_pool.tile([128, PC], F32, name="rec", tag="rec", bufs=2)
            nc.vector.reciprocal_approx_fast(out=rec[:], in_=s_ps[:])
            attn = wk_pool.tile([128, PC], BF16, name="attn", tag="attn",
                                bufs=4)
            nc.gpsimd.tensor_tensor(attn[:], expts[pc][:], rec[:],
                                    op=AluOpType.mult)
            attns[pc] = attn

        o_pss = [None] * NPC

        def stage_out_mm(pc):
            attn = attns[pc]
            # two double-bank PSUM tiles per chunk
            tiles = []
            for h in range(2):
                o_ps = ps.tile([128, 2, PC], F32, name=f"o_ps{pc}_{h}",
                               tag="o", bufs=2)
                for i in range(2):
                    ct = 2 * h + i
                    nc.tensor.matmul(o_ps[:, i, :],
                                     vstack[32 * ct:32 * (ct + 1), :],
                                     attn[32 * ct:32 * (ct + 1), :],
                                     start=True, stop=True,
                                     tile_position=(32 * ct, 0))
                tiles.append(o_ps)
            o_pss[pc] = tiles

        def stage_out_fin(pc):
            # finals run a full step after the matmuls so neither ACT nor
            # DVE ever waits on PSUM completion.
            # h0 detours through an ACT copy so its DVE add runs all-bf16
            # (2x_1p mode, ~half cost); h1 adds straight from PSUM.
            xt = xs[pc]
            osb = wk_pool.tile([128, CT, PC], BF16, name="osb", tag="osb",
                               bufs=3)
            tmp = wk_pool.tile([128, 2, PC], BF16, name="tmp",
                               tag="tmp", bufs=2)
            nc.scalar.activation(out=tmp[:], in_=o_pss[pc][0][:],
                                 func=AF.Copy, bias=0.0, scale=1.0)
            nc.vector.tensor_tensor(osb[:, 0:2, :], tmp[:],
                                    xt[:, 0:2, :], op=AluOpType.add)
            nc.vector.tensor_tensor(osb[:, 2:4, :], o_pss[pc][1][:],
                                    xt[:, 2:4, :], op=AluOpType.add)
            for h in range(2):
                dst = nc.t.out[2 * h * 128:(2 * h + 2) * 128,
                               pc * PC:(pc + 1) * PC].rearrange(
                    "(k p) f -> p k f", p=128)
                sync.dma_start(dst, osb[:, 2 * h:2 * h + 2, :])

        for step in range(NPC + 4):
            if 1 <= step and step + 3 < NPC:
                load_chunk(step + 3)
            if step < NPC:
                stage_e(step)
            if 0 <= step - 1 < NPC:
                stage_s(step - 1)
            if 0 <= step - 2 < NPC:
                stage_out_mm(step - 2)
            if 0 <= step - 3 < NPC:
                stage_out_fin(step - 3)


class _T:
    """Attribute access to declared dram params."""
    def __init__(self):
        self.__dict__ = {}


_NC_CACHE = []


def _build():
    if _NC_CACHE:
        return _NC_CACHE[0]
    nc = bacc.Bacc(target_bir_lowering=False)
    nc.t = _T()
    t = nc.t
    t.x = nc.declare_dram_parameter("x", [C, HW], BF16, isOutput=False)
    t.pem = nc.declare_dram_parameter("pem", [C, 128], BF16, isOutput=False)
    t.pvs = nc.declare_dram_parameter("pvs", [128, 130], BF16, isOutput=False)
    t.out = nc.declare_dram_parameter("out", [C, HW], BF16, isOutput=True)
    with tile.TileContext(nc) as tc:
        _emit(nc, tc)
    nc.finalize()
    _NC_CACHE.append(nc)
    return nc


def _in_maps(x, y, wq, bq, wk, bk, wv, bv, scale):
    x = np.ascontiguousarray(x, dtype=np.float32).reshape(N, C, HW).astype(BF)
    y = np.float32(y)
    wq, bq, wk, bk, wv, bv = (np.float32(a) for a in (wq, bq, wk, bk, wv, bv))
    s = float(np.float32(scale).reshape(-1)[0])
    maps = []
    for i in range(N):
        ktb = wk @ y[i].T + bk[:, None]              # [C4, K]
        ktb4 = np.tile(ktb, (1, 4))                  # [C4, 4K]
        em = (wq.T @ ktb4).astype(BF)                # [C, 4K=128]
        e_b4 = np.ascontiguousarray(
            (bq @ ktb4).astype(np.float32).reshape(128, 1))
        v = y[i] @ wv.T * s + s * bv                 # [K, C]
        vstack = np.ascontiguousarray(
            v.reshape(K, CT, 128).transpose(1, 0, 2).reshape(128, 128)
        ).astype(BF)                                 # [128, 128]
        pvs = np.concatenate([vstack, e_b4.view(BF)], axis=1)  # [128, 130]
        maps.append({"x": x[i], "pem": em, "pvs": pvs})
    return maps


def _run(inputs, **kwargs):
    nc = _build()
    return run_bass_kernel_spmd(nc, _in_maps(**inputs),
                                core_ids=list(range(N)), **kwargs)


def kernel(**inputs) -> np.ndarray:
    res = _run(inputs)
    out = np.stack([np.asarray(res.results[i]["out"], dtype=np.float32)
                    for i in range(N)])
    return out.reshape(N, C, H, W)


# revision 26
# speedup vs baseline: 1.0420x; 1.0420x over previous
"""CPAMDec attention-decoder kernel for 8 Trainium2 NeuronCores.

Reference computation (per batch n of N=8):
    q  = x_n^T @ wq^T + bq          (HW=4096, C4=128)
    k  = y_n @ wk^T + bk            (K=32, C4=128)
    v  = y_n @ wv^T + bv            (K=32, C=512)
    attn = softmax(q @ k^T, axis=-1)        (HW, K)
    out = scale * (v^T @ attn^T) + x_n      (C, HW)

Sharding: pure data parallel - core i computes batch i.

Everything that does not involve x (8MB/core) is precomputed on the host
in fp32 and shipped as two small consts (~170KB/core):
  - EM[c,j] = sum_o wq[o,c]*(k^T[o,j]+bk-bias'd)  so that per chunk
    e[j,p] = sum_c EM[c,j] x[c,p] + e_b[j]; e_b = bq @ ktb (exp bias).
  - vstack: scale*v (+ scale*bv baked into every row, valid since
    sum_j attn = 1) in the partition-stacked layout the 32-row
    tile_position matmuls want.

Device loop per 512-pixel chunk (bf16 end to end, rel-err ~2.7e-3 vs
the 2e-2 gate; HBM-per-NC ~358 GB/s is the roofline):
    e = EM^T x (4 mm) -> exp (ACT, bias e_b) -> sum via ones-matmul
    -> reciprocal (DVE) -> attn = expt*rec (Pool) -> 4x 32-row matmuls
    (PSUM, double-bank pairs) -> o_ps + x adds (DVE) -> store.
PE warm-up dummies ramp the HAM clock gate (1.2 -> 2.4 GHz after ~3.4us
of sustained execution) while the first DMAs land.
"""

import sys

sys.path.insert(0, "/opt/trn_rl_repo")

import numpy as np
import ml_dtypes

import concourse.bacc as bacc
import concourse.mybir as mybir
import concourse.tile as tile
from concourse.alu_op_type import AluOpType
from concourse.bass_utils import run_bass_kernel_spmd

F32 = mybir.dt.float32
BF16 = mybir.dt.bfloat16
AF = mybir.ActivationFunctionType
BF = ml_dtypes.bfloat16

N, C, H, W, K = 8, 512, 64, 64, 32
HW = H * W            # 4096
C4 = C // 4           # 128
PC = 512              # free-dim chunk (1 PSUM bank of fp32)
NPC = HW // PC        # 8 chunks
KC = C // 128         # 4 contraction chunks
CT = C // 128         # 4 output row-tiles


def _emit(nc, tc):
    sync = nc.sync

    with (
        tc.tile_pool(name="const", bufs=1) as cst,
        tc.tile_pool(name="xbuf", bufs=1) as xp,
        tc.tile_pool(name="work", bufs=3) as wk_pool,
        tc.tile_pool(name="ps", bufs=2, space="PSUM") as ps,
    ):
        # ---- consts first on the SP ring: FIFO beats the x flood ----
        # pem = EM as [512,128] -> [128, KC, 128]      bf16
        # pvs = vstack[128,128] | e_b (f32 as 2 bf16)  bf16
        pem = cst.tile([128, KC, 128], BF16, name="pem", tag="pem")
        sync.dma_start(pem[:],
                       nc.t.pem[:].rearrange("(k p) f -> p k f", p=128))
        pvs = cst.tile([128, 130], BF16, name="pvs", tag="pvs")
        sync.dma_start(pvs[:], nc.t.pvs[:])
        vstack = pvs[:, 0:128]
        e_b4 = pvs[:, 128:130].bitcast(F32)

        ones32 = cst.tile([K, 128], BF16, name="ones32", tag="ones32")
        nc.gpsimd.memset(ones32[:], 1.0)

        # x column chunks: (128 part, 4 c-tiles, PC cols) strided loads on
        # the SP ring, behind the consts. SBUF keeps all 8 resident.
        xs = [None] * NPC

        def load_chunk(pc):
            t = xp.tile([128, KC, PC], BF16, name=f"xs{pc}", tag=f"xs{pc}")
            src = nc.t.x[:, pc * PC:(pc + 1) * PC].rearrange(
                "(k p) f -> p k f", p=128)
            sync.dma_start(t[:], src)
            xs[pc] = t

        for pc in range(4):
            load_chunk(pc)

        # ---- PE warm-up: ramp the HAM clock gate while DMAs land ----
        dmy_ps = ps.tile([128, PC], F32, name="dmy_ps", tag="s", bufs=2)
        for _ in range(5):
            nc.tensor.matmul(dmy_ps[:], pem[:, 0, :], pem[:],
                             start=True, stop=True)

        # Load the exp ACT table before steady state (Copy/Identity live in
        # every table, so this is the only table load).
        acttbl = cst.tile([128, 8], BF16, name="acttbl", tag="acttbl")
        nc.scalar.activation(out=acttbl[:], in_=pem[:, 0, 0:8], func=AF.Exp,
                             bias=0.0, scale=1.0)

        # ------------- software-pipelined loop over column chunks ----
        #   step i:  e/exp(i)   sum/rec/mul(i-1)   out-mm/add/store(i-2)
        expts = [None] * NPC
        attns = [None] * NPC

        def stage_e(pc):
            e_ps = ps.tile([128, PC], F32, name=f"e_ps{pc}", tag="e", bufs=2)
            for k in range(KC):
                nc.tensor.matmul(e_ps[:], pem[:, k, :], xs[pc][:, k, :],
                                 start=(k == 0), stop=(k == KC - 1))
            expt = wk_pool.tile([128, PC], BF16, name="expt", tag="expt",
                                bufs=3)
            nc.scalar.activation(out=expt[:], in_=e_ps[:], func=AF.Exp,
                                 bias=e_b4[:], scale=1.0)
            expts[pc] = expt

        def stage_s(pc):
            s_ps = ps.tile([128, PC], F32, name=f"s_ps{pc}", tag="s", bufs=2)
            nc.tensor.matmul(s_ps[:], ones32[:], expts[pc][0:K, :],
                             start=True, stop=True)
            rec = wk_pool.tile([128, PC], F32, name="rec", tag="rec", bufs=2)
            nc.vector.reciprocal_approx_fast(out=rec[:], in_=s_ps[:])
            attn = wk_pool.tile([128, PC], BF16, name="attn", tag="attn",
                                bufs=4)
            nc.gpsimd.tensor_tensor(attn[:], expts[pc][:], rec[:],
                                    op=AluOpType.mult)
            attns[pc] = attn

        o_pss = [None] * NPC

        def stage_out_mm(pc):
            attn = attns[pc]
            # two double-bank PSUM tiles per chunk
            tiles = []
            for h in range(2):
                o_ps = ps.tile([128, 2, PC], F32, name=f"o_ps{pc}_{h}",
                               tag="o", bufs=2)
                for i in range(2):
                    ct = 2 * h + i
                    nc.tensor.matmul(o_ps[:, i, :],
                                     vstack[32 * ct:32 * (ct + 1), :],
                                     attn[32 * ct:32 * (ct + 1), :],
                                     start=True, stop=True,
                                     tile_position=(32 * ct, 0))
                tiles.append(o_ps)
            o_pss[pc] = tiles

        def stage_out_fin(pc):
            # finals run a full step after the matmuls so neither ACT nor
            # DVE ever waits on PSUM completion.
            # h0 detours through an ACT copy so its DVE add runs all-bf16
            # (2x_1p mode, ~half cost); h1 adds straight from PSUM.
            xt = xs[pc]
            osb = wk_pool.tile([128, CT, PC], BF16, name="osb", tag="osb",
                               bufs=3)
            tmp = wk_pool.tile([128, 2, PC], BF16, name="tmp",
                               tag="tmp", bufs=2)
            nc.scalar.activation(out=tmp[:], in_=o_pss[pc][0][:],
                                 func=AF.Copy, bias=0.0, scale=1.0)
            nc.vector.tensor_tensor(osb[:, 0:2, :], tmp[:],
                                    xt[:, 0:2, :], op=AluOpType.add)
            nc.vector.tensor_tensor(osb[:, 2:4, :], o_pss[pc][1][:],
                                    xt[:, 2:4, :], op=AluOpType.add)
            for h in range(2):
                dst = nc.t.out[2 * h * 128:(2 * h + 2) * 128,
                               pc * PC:(pc + 1) * PC].rearrange(
                    "(k p) f -> p k f", p=128)
                sync.dma_start(dst, osb[:, 2 * h:2 * h + 2, :])

        for step in range(NPC + 4):
            if 1 <= step and step + 3 < NPC:
                load_chunk(step + 3)
            if step < NPC:
                stage_e(step)
            if 0 <= step - 1 < NPC:
                stage_s(step - 1)
            if 0 <= step - 2 < NPC:
                stage_out_mm(step - 2)
            if 0 <= step - 3 < NPC:
                stage_out_fin(step - 3)


class _T:
    """Attribute access to declared dram params."""
    def __init__(self):
        self.__dict__ = {}


_NC_CACHE = []


def _build():
    if _NC_CACHE:
        return _NC_CACHE[0]
    nc = bacc.Bacc(target_bir_lowering=False)
    nc.t = _T()
    t = nc.t
    t.x = nc.declare_dram_parameter("x", [C, HW], BF16, isOutput=False)
    t.pem = nc.declare_dram_parameter("pem", [C, 128], BF16, isOutput=False)
    t.pvs = nc.declare_dram_parameter("pvs", [128, 130], BF16, isOutput=False)
    t.out = nc.declare_dram_parameter("out", [C, HW], BF16, isOutput=True)
    with tile.TileContext(nc) as tc:
        _emit(nc, tc)
    nc.finalize()
    _NC_CACHE.append(nc)
    return nc


def _in_maps(x, y, wq, bq, wk, bk, wv, bv, scale):
    x = np.ascontiguousarray(x, dtype=np.float32).reshape(N, C, HW).astype(BF)
    y = np.float32(y)
    wq, bq, wk, bk, wv, bv = (np.float32(a) for a in (wq, bq, wk, bk, wv, bv))
    s = float(np.float32(scale).reshape(-1)[0])
    maps = []
    for i in range(N):
        ktb = wk @ y[i].T + bk[:, None]              # [C4, K]
        ktb4 = np.tile(ktb, (1, 4))                  # [C4, 4K]
        em = (wq.T @ ktb4).astype(BF)                # [C, 4K=128]
        e_b4 = np.ascontiguousarray(
            (bq @ ktb4).astype(np.float32).reshape(128, 1))
        v = y[i] @ wv.T * s + s * bv                 # [K, C]
        vstack = np.ascontiguousarray(
            v.reshape(K, CT, 128).transpose(1, 0, 2).reshape(128, 128)
        ).astype(BF)                                 # [128, 128]
        pvs = np.concatenate([vstack, e_b4.view(BF)], axis=1)  # [128, 130]
        maps.append({"x": x[i], "pem": em, "pvs": pvs})
    return maps


def _run(inputs, **kwargs):
    nc = _build()
    return run_bass_kernel_spmd(nc, _in_maps(**inputs),
                                core_ids=list(range(N)), **kwargs)


def kernel(**inputs) -> np.ndarray:
    res = _run(inputs)
    out = np.stack([np.asarray(res.results[i]["out"], dtype=np.float32)
                    for i in range(N)])
    return out.reshape(N, C, H, W)
